# revision 3
# baseline (speedup 1.0000x reference)
"""MoE layer (8 experts, top-2) on 8 Trainium2 NeuronCores.

Strategy: expert parallelism with host-side dispatch + static load balance.
  - Host: gate logits (tiny matmul), top-2 + softmax, token->expert dispatch.
    The gate weight is folded into x (relu is positively homogeneous:
    relu(s*x@W1)@W2 = s*relu(x@W1)@W2 for s>0), so the device kernel is a
    pure two-layer FFN on pre-scaled tokens.
  - Load balance: instead of padding every core to the max expert count
    (2176 for the reference routing), each core runs three fixed-width
    slots (small, small, big; e.g. 112+144+1808 = 2064 columns).  A slot
    processes tokens of a single expert; a tiny solver assigns experts to
    the 24 slots so every expert's token count is covered.  All cores run
    the SAME program; only the DMA'd weights/tokens differ.
  - Device: for each slot, hT[f,c] = relu(w1T @ xT) then yT[d,c] = w2T @ hT.
    Layer 2 keeps tokens as the moving dim, so arbitrary (non-128) slot
    widths cost PE time proportional to width.  Weights are never resident:
    w1 streams as 2KB fc-chunks, w2 as 8KB dc-chunks through rotating tile
    pools, overlapped behind the matmul stream.
  - Host: out[token] += yT[:, col].T  (fp32 scatter-add over the two expert
    copies of each token).
"""

import os

os.environ.setdefault("BASS_NEVER_TRACE", "1")

import numpy as np
import ml_dtypes

D_MODEL = 1024
D_FF = 4096
NUM_EXPERTS = 8
TOP_K = 2
P = 128
KD = D_MODEL // P  # 8
KF = D_FF // P  # 32
C_BLK = 512

BF16 = ml_dtypes.bfloat16

_NC_CACHE: dict[tuple, object] = {}


# ---------------------------------------------------------------- solver ----
def solve_slots(counts, gran=16):
    """Choose per-core slot widths (sB, sC, s1) with sB<=sC<=512 small slots
    and one big slot, minimizing K = sB+sC+s1, such that the 8 copies of each
    slot can be assigned to experts with per-expert capacity >= count.
    Each expert is assumed to take exactly one big slot.
    Returns (widths, assign) where widths = (sB, sC, s1) and assign[e] =
    (nB, nC) small-slot counts for expert e.  Falls back to a single big
    slot at max count if the search fails."""
    import itertools
    from functools import lru_cache

    counts = [int(c) for c in counts]
    E = len(counts)
    K_base = max(-(-c // gran) * gran for c in counts)

    def try_structure(s1, sizes):
        res0 = [c - s1 for c in counts]
        order = sorted(range(E), key=lambda i: -res0[i])
        res = [res0[i] for i in order]
        m = len(sizes)

        def combos(r):
            out = []
            for cnt in itertools.product(*([range(E + 1)] * m)):
                tot = sum(c * s for c, s in zip(cnt, sizes))
                if tot >= r:
                    out.append((cnt, tot - r))
            out.sort(key=lambda x: x[1])
            keep = []
            for cnt, w in out:
                if not any(
                    all(cnt[i] >= k[i] for i in range(m)) and cnt != k
                    for k, _ in keep
                ):
                    keep.append((cnt, w))
            return keep[:64]

        opts = [combos(max(0, r)) for r in res]

        @lru_cache(maxsize=None)
        def dfs(idx, avail):
            if idx == E:
                return ()
            for cnt, w in opts[idx]:
                if all(cnt[i] <= avail[i] for i in range(m)):
                    rest = dfs(idx + 1, tuple(avail[i] - cnt[i] for i in range(m)))
                    if rest is not None:
                        return ((order[idx], cnt),) + rest
            return None

        return dfs(0, (E,) * len(sizes))

    for K in range(sum(counts) // E // gran * gran, K_base + gran, gran):
        best = None
        for sB in range(gran, 513, gran):
            for sC in range(sB, 513, gran):
                s1 = K - sB - sC
                if s1 < 1024:
                    continue
                sol = try_structure(s1, (sB, sC))
                if sol is not None:
                    if best is None or min(sB, sC) > min(best[0][:2]):
                        best = ((sB, sC, s1), sol)
        if best is not None:
            widths, sol = best
            assign = {e: cnt for e, cnt in sol}
            return widths, assign
    # fallback: one big slot per core, classic capacity padding
    return (0, 0, K_base), {e: (0, 0) for e in range(E)}


# --------------------------------------------------------------- program ----
def build_moe_nc(widths):
    """Bass/Tile program: per-core slots of the given widths (0-width slots
    skipped), each slot a 2-layer relu-FFN on its column range.

    DRAM inputs (per core), s indexes the non-zero slots:
      xs   [128, KD, K]        bf16  pre-scaled tokens: xs[p,k,c] = g_c*x[c,k*128+p]
      w1_s [128, KF, KD, 128]  bf16  w1_s[p,fc,k,j]  = w1[e_s][fc*128+j, k*128+p]
      w2_s [128, KD, KF, 128]  bf16  w2_s[p,dc,kf,j] = w2[e_s][dc*128+j, kf*128+p]
    DRAM output:
      y    [D, K] f32          y[d,c] = (relu(x_c@w1.T)@w2.T)[d]
    """
    import concourse.mybir as mybir
    import concourse.tile as tile
    from concourse import bacc

    bf16, f32 = mybir.dt.bfloat16, mybir.dt.float32
    K = sum(widths)
    slots = [w for w in widths if w > 0]

    nc = bacc.Bacc("TRN2", target_bir_lowering=False, debug=False)
    xs = nc.dram_tensor("xs", [P, KD, K], bf16, kind="ExternalInput")
    w1d = [
        nc.dram_tensor(f"w1_{j}", [P, KF, KD, P], bf16, kind="ExternalInput")
        for j in range(len(slots))
    ]
    w2d = [
        nc.dram_tensor(f"w2_{j}", [P, KD, KF, P], bf16, kind="ExternalInput")
        for j in range(len(slots))
    ]
    y = nc.dram_tensor("y", [D_MODEL, K], f32, kind="ExternalOutput")

    blocks = []  # (slot_idx, col_off, width)
    off = 0
    for j, W in enumerate(slots):
        o = 0
        while o < W:
            w = min(C_BLK, W - o)
            blocks.append((j, off + o, w))
            o += w
        off += W

    with tile.TileContext(nc) as tc:
        with (
            tc.tile_pool(name="w1pool", bufs=16) as w1pool,
            tc.tile_pool(name="w2pool", bufs=6) as w2pool,
            tc.tile_pool(name="xpool", bufs=3) as xpool,
            tc.tile_pool(name="hpool", bufs=2) as hpool,
            tc.tile_pool(name="ypool", bufs=4) as ypool,
            tc.tile_pool(name="phpool", bufs=3, space="PSUM") as phpool,
            tc.tile_pool(name="pypool", bufs=3, space="PSUM") as pypool,
        ):
            for j, off, w in blocks:
                xt = xpool.tile([P, KD, C_BLK], bf16, tag="xt")
                nc.sync.dma_start(xt[:, 0:4, :w], xs[:, 0:4, off : off + w])
                nc.sync.dma_start(xt[:, 4:8, :w], xs[:, 4:8, off : off + w])
                hT = hpool.tile([P, KF, C_BLK], bf16, tag="hT")
                for fc in range(KF):
                    w1c = w1pool.tile([P, KD, P], bf16, tag="w1c")
                    nc.sync.dma_start(w1c[:], w1d[j][:, fc])
                    ph = phpool.tile([P, C_BLK], f32, tag="ph")
                    for k in range(KD):
                        nc.tensor.matmul(
                            ph[:, :w],
                            lhsT=w1c[:, k],
                            rhs=xt[:, k, :w],
                            start=(k == 0),
                            stop=(k == KD - 1),
                        )
                    nc.vector.tensor_scalar_max(hT[:, fc, :w], ph[:, :w], 0.0)
                for dc in range(KD):
                    w2c = w2pool.tile([P, KF, P], bf16, tag="w2c")
                    nc.sync.dma_start(w2c[:], w2d[j][:, dc])
                    py = pypool.tile([P, C_BLK], f32, tag="py")
                    for kf in range(KF):
                        nc.tensor.matmul(
                            py[:, :w],
                            lhsT=w2c[:, kf],
                            rhs=hT[:, kf, :w],
                            start=(kf == 0),
                            stop=(kf == KF - 1),
                        )
                    ys = ypool.tile([P, C_BLK], f32, tag="ys")
                    nc.scalar.copy(ys[:, :w], py[:, :w])
                    nc.sync.dma_start(y[dc * P : (dc + 1) * P, off : off + w], ys[:, :w])

    nc.compile()
    return nc


# ------------------------------------------------------------------ host ----
def route_tokens(xf: np.ndarray, gate_w: np.ndarray):
    """Top-2 routing, replicating jax.lax.top_k tie-breaking (lowest index)."""
    logits = xf @ gate_w.astype(np.float32).T  # [T, E]
    top2 = np.argsort(-logits, axis=-1, kind="stable")[:, :TOP_K]
    tv = np.take_along_axis(logits, top2, axis=-1)
    tv = tv - tv.max(axis=-1, keepdims=True)
    ex = np.exp(tv)
    gates = ex / ex.sum(axis=-1, keepdims=True)
    rows, weights = [], []
    for e in range(NUM_EXPERTS):
        r, kpos = np.nonzero(top2 == e)
        rows.append(r)
        weights.append(gates[r, kpos].astype(np.float32))
    return rows, weights


def _w_layouts(w1, w2):
    """Per-expert DRAM weight layouts (cached per kernel() call)."""
    w1L, w2L = [], []
    for e in range(NUM_EXPERTS):
        W1 = w1[e].astype(BF16)  # [F, D]
        w1L.append(
            np.ascontiguousarray(W1.reshape(KF, P, KD, P).transpose(3, 0, 2, 1))
        )  # [p, fc, k, j]
        W2 = w2[e].astype(BF16)  # [D, F]
        w2L.append(
            np.ascontiguousarray(W2.reshape(KD, P, KF, P).transpose(3, 0, 2, 1))
        )  # [p, dc, kf, j]
    return w1L, w2L


def kernel(x, gate_w, w1, w2):
    from concourse.bass_utils import run_bass_kernel_spmd

    x = np.asarray(x)
    gate_w = np.asarray(gate_w)
    w1 = np.asarray(w1)
    w2 = np.asarray(w2)
    B, S, D = x.shape

    xf = x.reshape(-1, D).astype(np.float32)
    rows, weights = route_tokens(xf, gate_w)
    counts = [len(r) for r in rows]

    widths, assign = solve_slots(counts)
    sB, sC, s1 = widths
    slots = [w for w in widths if w > 0]
    slot_pos = {}  # program slot index per structural slot (B, C, big)
    si = 0
    for name, w in zip(("B", "C", "big"), widths):
        if w > 0:
            slot_pos[name] = (si, w)
            si += 1

    # --- assign experts to the 8 copies of each slot ---------------------
    # slot table: per core, per program-slot -> expert (or None)
    n_slots = len(slots)
    core_slot_expert = [[None] * n_slots for _ in range(NUM_EXPERTS)]
    # big slot: expert e on core e
    if "big" in slot_pos:
        bi = slot_pos["big"][0]
        for e in range(NUM_EXPERTS):
            core_slot_expert[e][bi] = e
    # small slots: hand out inventory in core order
    for name in ("B", "C"):
        if name not in slot_pos:
            continue
        siB = slot_pos[name][0]
        free = list(range(NUM_EXPERTS))  # cores with this slot unassigned
        for e in range(NUM_EXPERTS):
            nB, nC = assign.get(e, (0, 0))
            need = nB if name == "B" else nC
            for _ in range(need):
                core = free.pop(0)
                core_slot_expert[core][siB] = e

    # --- fill tokens into slots ------------------------------------------
    # per expert: list of (core, slot_idx, width) big first then smalls
    slot_offsets = np.concatenate([[0], np.cumsum(slots)])[:-1]
    expert_slots = {e: [] for e in range(NUM_EXPERTS)}
    order_names = [n for n in ("big", "B", "C") if n in slot_pos]
    for name in order_names:
        si_, w_ = slot_pos[name]
        for core in range(NUM_EXPERTS):
            e = core_slot_expert[core][si_]
            if e is not None:
                expert_slots[e].append((core, si_, w_))

    # columns: for each (core, slot): (expert, token_ids)
    fills = {}  # (core, slot_idx) -> token index array
    for e in range(NUM_EXPERTS):
        toks = rows[e]
        gws = weights[e]
        pos = 0
        for core, si_, w_ in expert_slots[e]:
            take = min(w_, len(toks) - pos)
            if take < 0:
                take = 0
            fills[(core, si_)] = (toks[pos : pos + take], gws[pos : pos + take])
            pos += take
        assert pos >= len(toks), (
            f"expert {e}: {len(toks)} tokens, capacity "
            f"{sum(w for _, _, w in expert_slots[e])}"
        )

    # --- build per-core inputs -------------------------------------------
    w1L, w2L = _w_layouts(w1, w2)
    K = sum(slots)
    in_maps = []
    for core in range(NUM_EXPERTS):
        xs = np.zeros((P, KD, K), BF16)
        for si_ in range(n_slots):
            toks, gws = fills.get((core, si_), (np.array([], np.int64), None))
            cnt = len(toks)
            if cnt:
                blk = xf[toks] * gws[:, None]  # [cnt, D] f32, gate folded in
                blk = blk.astype(BF16).T.reshape(KD, P, cnt).transpose(1, 0, 2)
                off = slot_offsets[si_]
                xs[:, :, off : off + cnt] = blk
        im = {"xs": np.ascontiguousarray(xs)}
        for si_ in range(n_slots):
            e = core_slot_expert[core][si_]
            if e is None:
                e = 0  # unused slot: any weights; its columns are zero
            im[f"w1_{si_}"] = w1L[e]
            im[f"w2_{si_}"] = w2L[e]
        in_maps.append(im)

    key = tuple(slots)
    nc = _NC_CACHE.get(key)
    if nc is None:
        nc = _NC_CACHE[key] = build_moe_nc(widths)
    res = run_bass_kernel_spmd(nc, in_maps, core_ids=list(range(NUM_EXPERTS)))

    out = np.zeros((B * S, D), np.float32)
    for core in range(NUM_EXPERTS):
        yT = res.results[core]["y"]  # [D, K] f32
        for si_ in range(n_slots):
            toks, _ = fills.get((core, si_), (np.array([], np.int64), None))
            cnt = len(toks)
            if cnt:
                off = slot_offsets[si_]
                # tokens are unique within a slot (one copy per expert), so
                # fancy-index += is safe and much faster than np.add.at
                out[toks] += yT[:, off : off + cnt].T
    return out.reshape(B, S, D)


# revision 4
# speedup vs baseline: 1.1055x; 1.1055x over previous
"""MoE layer (8 experts, top-2) on 8 Trainium2 NeuronCores.

Strategy: expert parallelism with host-side dispatch + static load balance.
  - Host: gate logits (tiny matmul), top-2 + softmax, token->expert dispatch.
    The gate weight is folded into x (relu is positively homogeneous:
    relu(s*x@W1)@W2 = s*relu(x@W1)@W2 for s>0), so the device kernel is a
    pure two-layer FFN on pre-scaled tokens.
  - Load balance: instead of padding every core to the max expert count
    (2176 for the reference routing), each core runs four fixed-width slots
    (e.g. 512+512+512+544 = 2080 columns).  A slot processes tokens of a
    single expert; a tiny solver assigns experts to the 32 slots so every
    expert's token count is covered.  All cores run the SAME program; only
    the DMA'd weights/tokens differ.  Slots are all >= 512 wide so each
    slot's compute (~109us) covers its own weight stream (~48us on the
    shared DMA path).
  - Device, per slot: hT[f,c] = relu(w1T @ xT), then yT[d,c] = w2T @ hT.
    Layer 2 keeps tokens as the moving dim, so arbitrary (non-128) slot
    widths cost PE time proportional to width.  Weights are never resident:
    w1 streams once per slot as 2KB fc-chunks, w2 as 8KB dc-chunks, through
    rotating tile pools overlapped behind the matmul stream.
  - Host: out[token] += yT[:, cols].T  (fp32 combine of the two expert
    copies of each token).
"""

import os

os.environ.setdefault("BASS_NEVER_TRACE", "1")

import numpy as np
import ml_dtypes

D_MODEL = 1024
D_FF = 4096
NUM_EXPERTS = 8
TOP_K = 2
P = 128
KD = D_MODEL // P  # 8
KF = D_FF // P  # 32
C_BLK = 512

BF16 = ml_dtypes.bfloat16

_NC_CACHE: dict[tuple, object] = {}


# ---------------------------------------------------------------- solver ----
def solve_slots(counts, gran=16):
    """Choose per-core slot widths, all >= 512 (so each slot's compute hides
    its own weight stream), minimizing K = sum(widths), such that the 8
    copies of each width can be assigned to experts with per-expert capacity
    >= token count.  Tokens of one expert may span slots on any cores.

    Returns (widths, assign): widths is the per-core slot tuple; assign[e]
    is a tuple of per-width slot counts for expert e."""
    import itertools
    from functools import lru_cache

    counts = [int(c) for c in counts]
    E = len(counts)
    total = sum(counts)
    K_max = max(-(-c // gran) * gran for c in counts)

    def feasible(sizes, inv):
        """sizes: distinct slot widths; inv: copies of each available.
        Experts may take any multiset; returns per-expert counts or None."""
        order = sorted(range(E), key=lambda i: -counts[i])
        m = len(sizes)

        def combos(r):
            out = []
            for cnt in itertools.product(*[range(v + 1) for v in inv]):
                tot = sum(c * s for c, s in zip(cnt, sizes))
                if tot >= r:
                    out.append((cnt, tot - r))
            out.sort(key=lambda x: x[1])
            keep = []
            for cnt, w in out:
                if not any(
                    all(cnt[i] >= k[i] for i in range(m)) and cnt != k
                    for k, _ in keep
                ):
                    keep.append((cnt, w))
            return keep[:64]

        opts = [combos(counts[i]) for i in order]

        @lru_cache(maxsize=None)
        def dfs(idx, avail):
            if idx == E:
                return ()
            for cnt, w in opts[idx]:
                if all(cnt[i] <= avail[i] for i in range(m)):
                    rest = dfs(idx + 1, tuple(avail[i] - cnt[i] for i in range(m)))
                    if rest is not None:
                        return ((order[idx], cnt),) + rest
            return None

        return dfs(0, tuple(inv))

    # n slots per core, widths (a, b, 512, 512, ...) with a >= b >= 512
    best = None
    for K in range(-(-total // (E * gran)) * gran, K_max + gran, gran):
        for n in (4, 3, 5):
            base = 512 * (n - 2)
            for b in range(512, K - base - 512 + 1, gran):
                a = K - base - b
                if a < b:
                    break
                if n == 3 and a > 2 * b:
                    continue
                sizes, inv = [], []
                for s, c in ((a, 1), (b, 1), (512, n - 2)):
                    if c == 0:
                        continue
                    if sizes and s == sizes[-1]:
                        inv[-1] += c * E
                    else:
                        sizes.append(s)
                        inv.append(c * E)
                sol = feasible(tuple(sizes), tuple(inv))
                if sol is not None:
                    widths = tuple([512] * (n - 2) + [b, a])
                    assign = {}
                    for e, cnt in sol:
                        assign[e] = (tuple(sizes), cnt)
                    best = (widths, assign)
                    break
            if best:
                break
        if best:
            break
    if best is not None:
        return best
    # fallback: one big slot per core, classic capacity padding
    return (K_max,), {e: ((K_max,), (1,)) for e in range(E)}


# --------------------------------------------------------------- program ----
def build_moe_nc(widths):
    """Bass/Tile program: per-core slots of the given widths, each slot a
    2-layer relu-FFN on its column range, weights streamed once per slot.

    DRAM inputs (per core), s indexes slots:
      xs   [128, KD, K]        bf16  pre-scaled tokens: xs[p,k,c] = g_c*x[c,k*128+p]
      w1_s [128, KF, KD, 128]  bf16  w1_s[p,fc,k,j]  = w1[e_s][fc*128+j, k*128+p]
      w2_s [128, KD, KF, 128]  bf16  w2_s[p,dc,kf,j] = w2[e_s][dc*128+j, kf*128+p]
    DRAM output:
      y    [D, K] f32          y[d,c] = (relu(x_c@w1.T)@w2.T)[d]
    """
    import concourse.mybir as mybir
    import concourse.tile as tile
    from concourse import bacc

    bf16, f32 = mybir.dt.bfloat16, mybir.dt.float32
    slots = list(widths)
    K = sum(slots)
    Wmax = max(slots)

    nc = bacc.Bacc("TRN2", target_bir_lowering=False, debug=False)
    xs = nc.dram_tensor("xs", [P, KD, K], bf16, kind="ExternalInput")
    w1d = [
        nc.dram_tensor(f"w1_{j}", [P, KF, KD, P], bf16, kind="ExternalInput")
        for j in range(len(slots))
    ]
    w2d = [
        nc.dram_tensor(f"w2_{j}", [P, KD, KF, P], bf16, kind="ExternalInput")
        for j in range(len(slots))
    ]
    y = nc.dram_tensor("y", [D_MODEL, K], f32, kind="ExternalOutput")

    with tile.TileContext(nc) as tc:
        with (
            tc.tile_pool(name="w1pool", bufs=8) as w1pool,
            tc.tile_pool(name="w2pool", bufs=4) as w2pool,
            tc.tile_pool(name="xpool", bufs=2) as xpool,
            tc.tile_pool(name="hpool", bufs=2) as hpool,
            tc.tile_pool(name="ypool", bufs=4) as ypool,
            tc.tile_pool(name="phpool", bufs=3, space="PSUM") as phpool,
            tc.tile_pool(name="pypool", bufs=3, space="PSUM") as pypool,
        ):
            off = 0
            for j, W in enumerate(slots):
                # sub-blocks of <= 512 cols (PSUM bank width)
                sub, o = [], 0
                while o < W:
                    cw = min(C_BLK, W - o)
                    sub.append((o, cw))
                    o += cw
                xt = xpool.tile([P, KD, Wmax], bf16, tag="xt")
                for k0 in range(0, KD, 2):
                    nc.sync.dma_start(
                        xt[:, k0 : k0 + 2, :W], xs[:, k0 : k0 + 2, off : off + W]
                    )
                hT = hpool.tile([P, KF, Wmax], bf16, tag="hT")
                for fc in range(KF):
                    w1c = w1pool.tile([P, KD, P], bf16, tag="w1c")
                    nc.sync.dma_start(w1c[:], w1d[j][:, fc])
                    for co, cw in sub:
                        ph = phpool.tile([P, C_BLK], f32, tag="ph")
                        for k in range(KD):
                            nc.tensor.matmul(
                                ph[:, :cw],
                                lhsT=w1c[:, k],
                                rhs=xt[:, k, co : co + cw],
                                start=(k == 0),
                                stop=(k == KD - 1),
                            )
                        nc.vector.tensor_scalar_max(
                            hT[:, fc, co : co + cw], ph[:, :cw], 0.0
                        )
                for dc in range(KD):
                    w2c = w2pool.tile([P, KF, P], bf16, tag="w2c")
                    nc.sync.dma_start(w2c[:], w2d[j][:, dc])
                    for co, cw in sub:
                        py = pypool.tile([P, C_BLK], f32, tag="py")
                        for kf in range(KF):
                            nc.tensor.matmul(
                                py[:, :cw],
                                lhsT=w2c[:, kf],
                                rhs=hT[:, kf, co : co + cw],
                                start=(kf == 0),
                                stop=(kf == KF - 1),
                            )
                        ys = ypool.tile([P, C_BLK], f32, tag="ys")
                        nc.scalar.copy(ys[:, :cw], py[:, :cw])
                        nc.sync.dma_start(
                            y[dc * P : (dc + 1) * P, off + co : off + co + cw],
                            ys[:, :cw],
                        )
                off += W

    nc.compile()
    return nc


# ------------------------------------------------------------------ host ----
def route_tokens(xf: np.ndarray, gate_w: np.ndarray):
    """Top-2 routing, replicating jax.lax.top_k tie-breaking (lowest index)."""
    logits = xf @ gate_w.astype(np.float32).T  # [T, E]
    top2 = np.argsort(-logits, axis=-1, kind="stable")[:, :TOP_K]
    tv = np.take_along_axis(logits, top2, axis=-1)
    tv = tv - tv.max(axis=-1, keepdims=True)
    ex = np.exp(tv)
    gates = ex / ex.sum(axis=-1, keepdims=True)
    rows, weights = [], []
    for e in range(NUM_EXPERTS):
        r, kpos = np.nonzero(top2 == e)
        rows.append(r)
        weights.append(gates[r, kpos].astype(np.float32))
    return rows, weights


def _w_layouts(w1, w2):
    """Per-expert DRAM weight layouts."""
    w1L, w2L = [], []
    for e in range(NUM_EXPERTS):
        W1 = w1[e].astype(BF16)  # [F, D]
        w1L.append(
            np.ascontiguousarray(W1.reshape(KF, P, KD, P).transpose(3, 0, 2, 1))
        )  # [p, fc, k, j]
        W2 = w2[e].astype(BF16)  # [D, F]
        w2L.append(
            np.ascontiguousarray(W2.reshape(KD, P, KF, P).transpose(3, 0, 2, 1))
        )  # [p, dc, kf, j]
    return w1L, w2L


def kernel(x, gate_w, w1, w2):
    from concourse.bass_utils import run_bass_kernel_spmd

    x = np.asarray(x)
    gate_w = np.asarray(gate_w)
    w1 = np.asarray(w1)
    w2 = np.asarray(w2)
    B, S, D = x.shape

    xf = x.reshape(-1, D).astype(np.float32)
    rows, weights = route_tokens(xf, gate_w)
    counts = [len(r) for r in rows]

    widths, assign = solve_slots(counts)
    slots = list(widths)
    n_slots = len(slots)
    slot_offsets = np.concatenate([[0], np.cumsum(slots)])[:-1]

    # --- assign experts to the 8 copies of each slot ---------------------
    # inventory: per width-value, list of (core, slot_idx) free copies
    from collections import defaultdict

    free = defaultdict(list)
    for core in range(NUM_EXPERTS):
        for si in range(n_slots):
            free[slots[si]].append((core, si))
    core_slot_expert = [[None] * n_slots for _ in range(NUM_EXPERTS)]
    expert_slots = {e: [] for e in range(NUM_EXPERTS)}
    # larger experts first so they grab contiguous inventory
    for e in sorted(range(NUM_EXPERTS), key=lambda e: -counts[e]):
        sizes, cnt = assign[e]
        for s, c in zip(sizes, cnt):
            for _ in range(c):
                core, si = free[s].pop(0)
                core_slot_expert[core][si] = e
                expert_slots[e].append((core, si, s))

    # --- fill tokens into slots ------------------------------------------
    fills = {}  # (core, slot_idx) -> (token_ids, gate_weights)
    for e in range(NUM_EXPERTS):
        toks, gws = rows[e], weights[e]
        pos = 0
        for core, si, w_ in expert_slots[e]:
            take = max(0, min(w_, len(toks) - pos))
            fills[(core, si)] = (toks[pos : pos + take], gws[pos : pos + take])
            pos += take
        assert pos >= len(toks), (
            f"expert {e}: {len(toks)} tokens, capacity "
            f"{sum(w for _, _, w in expert_slots[e])}"
        )

    # --- build per-core inputs -------------------------------------------
    w1L, w2L = _w_layouts(w1, w2)
    K = sum(slots)
    in_maps = []
    for core in range(NUM_EXPERTS):
        xs = np.zeros((P, KD, K), BF16)
        for si in range(n_slots):
            toks, gws = fills.get((core, si), (np.array([], np.int64), None))
            cnt = len(toks)
            if cnt:
                blk = xf[toks] * gws[:, None]  # [cnt, D] f32, gate folded in
                blk = blk.astype(BF16).T.reshape(KD, P, cnt).transpose(1, 0, 2)
                off = slot_offsets[si]
                xs[:, :, off : off + cnt] = blk
        im = {"xs": np.ascontiguousarray(xs)}
        for si in range(n_slots):
            e = core_slot_expert[core][si]
            if e is None:
                e = 0  # unused slot: any weights; its columns are zero
            im[f"w1_{si}"] = w1L[e]
            im[f"w2_{si}"] = w2L[e]
        in_maps.append(im)

    key = tuple(slots)
    nc = _NC_CACHE.get(key)
    if nc is None:
        nc = _NC_CACHE[key] = build_moe_nc(key)
    res = run_bass_kernel_spmd(nc, in_maps, core_ids=list(range(NUM_EXPERTS)))

    out = np.zeros((B * S, D), np.float32)
    for core in range(NUM_EXPERTS):
        yT = res.results[core]["y"]  # [D, K] f32
        for si in range(n_slots):
            toks, _ = fills.get((core, si), (np.array([], np.int64), None))
            cnt = len(toks)
            if cnt:
                off = slot_offsets[si]
                # tokens are unique within a slot (one copy per expert), so
                # fancy-index += is safe and much faster than np.add.at
                out[toks] += yT[:, off : off + cnt].T
    return out.reshape(B, S, D)
